# revision 27
# baseline (speedup 1.0000x reference)
"""Trainium2 Bass kernel for nn_Expand_36610301231376.

kernel(**inputs) takes the FULL unsharded inputs (as in reference.setup_inputs)
and returns the FULL (16, 512, 56, 56) float32 output.

Strategy: pure data parallel over batch B=16 across 8 NeuronCores (2 batches
per core); all parameters replicated. Per core, tokens (56x56=3136) are
processed channel-major in 7 chunks of 448 (8 image rows); row-local attention
runs on 2-row blocks of 112 tokens.

Key structure (v2):
- All elementwise traffic is bf16 (y, consts, output) for 2x/4x DVE modes;
  output is upcast to f32 on host.
- LayerNorm statistics: channel-dim sums via ones-matmuls into [1,448] PSUM
  rows, then tiny PE transposes to [112,4] column form so sqrt/reciprocal run
  parallel across partitions (the serial [1,448] reciprocal was ~3.7us each).
- LN gammas fold into the projection weights; LN1's r folds into a
  per-partition tensor_scalar on the score PSUM; the conv bias cancels in LN.
- The softmax mask is a rank-3 matmul accumulated into the score PSUM.
- v comes from a host-supplied token-major copy of y normalized with
  per-partition tensor_scalar ops (no PE transposes of ny); the positional-
  encoding contribution to the attention output is two extra const-lhsT
  matmuls accumulated into the same PSUM group.
"""
import sys

if "/opt/trn_rl_repo" not in sys.path:
    sys.path.insert(0, "/opt/trn_rl_repo")

import numpy as np
import orjson

# ----------------------------------------------------------------------------
# BIR post-pass: this container's walrus build supports only ONE sync-wait per
# instruction; split multi-wait instructions into single-wait NoOps.
# ----------------------------------------------------------------------------
_wcounter = [0]


def _split_block(instructions):
    out, changed = [], False
    for inst in instructions:
        si = inst.get("sync_info")
        waits = (si or {}).get("on_wait") or []
        if len(waits) > 1:
            changed = True
            for w in waits[:-1]:
                _wcounter[0] += 1
                nop = {
                    "engine": inst["engine"], "ins": [], "outs": [],
                    "name": f"I-wsplit-{_wcounter[0]}", "opcode": "NoOp",
                    "sync_info": {"on_update": [], "on_wait": [w]},
                }
                if "debug" in inst:
                    nop["debug"] = inst["debug"]
                out.append(nop)
            si["on_wait"] = [waits[-1]]
        out.append(inst)
    return out, changed


def _split_multi_waits_json(bir_json: bytes) -> bytes:
    m = orjson.loads(bir_json)
    changed = False
    for fn in m.get("functions", []):
        for blk in fn.get("blocks", []):
            insts = blk.get("instructions")
            if insts:
                blk["instructions"], ch = _split_block(insts)
                changed = changed or ch
    return orjson.dumps(m) if changed else bir_json


def _install_patch():
    import concourse.bass as bass

    if getattr(bass.Bass, "_wait_split_installed", False):
        return
    orig = bass.Bass.to_json_bytes

    def to_json_bytes(self):
        return _split_multi_waits_json(orig(self))

    bass.Bass.to_json_bytes = to_json_bytes
    bass.Bass._wait_split_installed = True


# ----------------------------------------------------------------------------
# Problem constants (hardcoded from the problem spec)
# ----------------------------------------------------------------------------
B = 16
N_CORES = 8
B_LOC = B // N_CORES
T_LEN, T_DIM = 149, 768
H = W = 56
S_DIM = 512
N_TOK = H * W           # 3136
CH = 448                # tokens per chunk (8 image rows)
NCHUNK = N_TOK // CH    # 7
NBLK = CH // 112        # 4 two-row attention blocks per chunk
EPS = 1e-5
BIGNEG = -1e30


# ----------------------------------------------------------------------------
# Device program
# ----------------------------------------------------------------------------
def _build_program():
    import concourse.bass as bass
    import concourse.tile as tile
    from concourse import mybir

    F32 = mybir.dt.float32
    BF16 = mybir.dt.bfloat16
    AF = mybir.ActivationFunctionType
    OP = mybir.AluOpType

    nc = bass.Bass(trn_type="TRN2", target_bir_lowering=False, debug=False)
    din = {}
    for name, shape, dt_ in [
        ("x0", (128, B_LOC, T_DIM), BF16), ("x1", (32, B_LOC, T_DIM), BF16),
        ("w1t", (128, 2, N_TOK), BF16),
        ("wqgt", (128, 6, S_DIM), BF16), ("uq", (1, S_DIM), BF16),
        ("cq", (128, 4, N_TOK), BF16),
        ("wkgt", (128, 4, S_DIM), BF16), ("ck", (128, 4, N_TOK), BF16),
        ("gb", (112, S_DIM), BF16),
        ("mA", (3, 112), BF16), ("mB", (3, 112), BF16),
        ("onesc", (128, 1), BF16), ("ones112s", (112, 128), BF16),
        ("ones112q", (112, 128), BF16), ("ones112m", (112, 128), BF16),
        ("id112", (112, 112), BF16), ("idf", (2, 2), F32),
        ("y", (B_LOC, 128, 4, N_TOK), BF16),
        ("ytm", (B_LOC, NCHUNK, 112, NBLK, S_DIM), BF16),
        ("petm", (NCHUNK, 112, NBLK, S_DIM), BF16),
    ]:
        din[name] = nc.dram_tensor(name, list(shape), dt_, kind="ExternalInput").ap()
    dout = nc.dram_tensor("out", [B_LOC, 128, 4, N_TOK], BF16,
                          kind="ExternalOutput").ap()

    from contextlib import ExitStack

    with nc.allow_low_precision(reason="bf16 ops, fp32 accumulate"), \
         tile.TileContext(nc) as tc, ExitStack() as ctx:
        singles = ctx.enter_context(tc.tile_pool(name="singles", bufs=1))
        io3 = ctx.enter_context(tc.tile_pool(name="io3", bufs=3))
        io2 = ctx.enter_context(tc.tile_pool(name="io2", bufs=2))
        outp = ctx.enter_context(tc.tile_pool(name="outp", bufs=2))
        wk = ctx.enter_context(tc.tile_pool(name="wk", bufs=2))
        sc = ctx.enter_context(tc.tile_pool(name="sc", bufs=2))
        att = ctx.enter_context(tc.tile_pool(name="att", bufs=3))
        ps_mm = ctx.enter_context(tc.tile_pool(name="ps_mm", bufs=3, space="PSUM"))
        ps_st = ctx.enter_context(tc.tile_pool(name="ps_st", bufs=2, space="PSUM"))
        ps_sm = ctx.enter_context(tc.tile_pool(name="ps_sm", bufs=1, space="PSUM"))
        ps_att = ctx.enter_context(tc.tile_pool(name="ps_att", bufs=2, space="PSUM"))

        def load(name, shape, dt_):
            t = singles.tile(list(shape), dt_, tag=name)
            nc.sync.dma_start(out=t, in_=din[name])
            return t

        x0 = load("x0", (128, B_LOC, T_DIM), BF16)
        x1 = load("x1", (32, B_LOC, T_DIM), BF16)
        onesc = load("onesc", (128, 1), BF16)
        ones112s = load("ones112s", (112, 128), BF16)
        ones112q = load("ones112q", (112, 128), BF16)
        ones112m = load("ones112m", (112, 128), BF16)
        id112 = load("id112", (112, 112), BF16)
        idf = load("idf", (2, 2), F32)
        mA = load("mA", (3, 112), BF16)
        mB = load("mB", (3, 112), BF16)
        gb = load("gb", (112, S_DIM), BF16)
        uq = load("uq", (1, S_DIM), BF16)

        epsb1 = singles.tile([112, 1], F32, tag="epsb1")
        nc.vector.memset(epsb1, T_DIM * EPS)
        epsv = singles.tile([112, 1], F32, tag="epsv")
        nc.vector.memset(epsv, EPS)

        x_k = [(x0, 128), (x1, 21)]

        # per-batch column sums of x (for the LN1 sum row): xbar_l = sum_d x[l,d]
        junk0 = singles.tile([128, T_DIM], BF16, tag="junk0")
        junk1 = singles.tile([32, T_DIM], BF16, tag="junk1")
        xb0f = singles.tile([128, B_LOC], F32, tag="xb0f")
        xb1f = singles.tile([32, B_LOC], F32, tag="xb1f")
        for b in range(B_LOC):
            nc.scalar.activation(out=junk0, in_=x0[:, b, :], func=AF.Copy,
                                 accum_out=xb0f[:, b:b + 1])
            nc.scalar.activation(out=junk1, in_=x1[:, b, :], func=AF.Copy,
                                 accum_out=xb1f[:, b:b + 1])
        xb0 = singles.tile([128, B_LOC], BF16, tag="xb0")
        nc.vector.tensor_copy(out=xb0, in_=xb0f)
        xb1 = singles.tile([32, B_LOC], BF16, tag="xb1")
        nc.vector.tensor_copy(out=xb1, in_=xb1f)

        def phase_load(st):
            b, ic = st["b"], st["ic"]
            ybf = io3.tile([128, 4, CH], BF16, tag="ybf")
            nc.sync.dma_start(out=ybf, in_=din["y"][b, :, :, st["cols"]])
            ytm = io3.tile([112, NBLK, S_DIM], BF16, tag="ytm")
            nc.sync.dma_start(out=ytm, in_=din["ytm"][b, ic])
            st["ybf"], st["ytm"] = ybf, ytm

        def _stat_cols(ps_s, ps_q, d, epsb):
            """[1,CH] PSUM sum/sumsq rows -> column form [112,4] stats."""
            rows_s = sc.tile([1, CH], F32, tag="rows_s")
            nc.vector.tensor_copy(out=rows_s, in_=ps_s)
            rows_q = sc.tile([1, CH], F32, tag="rows_q")
            nc.scalar.activation(out=rows_q, in_=ps_q, func=AF.Copy)
            ps_c = ps_sm.tile([112, 8], F32, tag="col")
            id1 = idf[0:1, 0:1]
            for blk in range(NBLK):
                tb = slice(blk * 112, (blk + 1) * 112)
                nc.tensor.transpose(ps_c[:, blk:blk + 1], rows_s[:, tb], id1)
                nc.tensor.transpose(ps_c[:, 4 + blk:5 + blk], rows_q[:, tb], id1)
            scol = ps_c[:, 0:4]
            qcol = ps_c[:, 4:8]
            m2c = sc.tile([112, 4], F32, tag="m2c")
            nc.scalar.activation(out=m2c, in_=scol, func=AF.Square,
                                 scale=float(1.0 / np.sqrt(d)))
            uc = sc.tile([112, 4], F32, tag="uc")
            nc.vector.tensor_tensor(out=uc, in0=qcol, in1=m2c, op=OP.subtract)
            sg = sc.tile([112, 4], F32, tag="sg")
            nc.scalar.activation(out=sg, in_=uc, func=AF.Sqrt, bias=epsb)
            return scol, sg

        def _diag_prep(colap, tag):
            dg = sc.tile([112, NBLK, 112], BF16, tag=tag)
            for blk in range(NBLK):
                nc.vector.tensor_scalar(
                    out=dg[:, blk, :], in0=id112,
                    scalar1=colap[:, blk:blk + 1],
                    scalar2=None, op0=OP.mult)
            return dg

        def _diag_bcast(prb, colap, ones_t, dg=None):
            if dg is None:
                dg = _diag_prep(colap, "dgr")
            for blk in range(NBLK):
                tb = slice(blk * 112, (blk + 1) * 112)
                nc.tensor.matmul(prb[:, tb], ones_t, dg[:, blk, :],
                                 start=True, stop=True)

        def front_a(st):
            """LN2 stats via bn_stats/bn_aggr on token-major y (free-axis)."""
            ytm = st["ytm"]
            bn = sc.tile([112, NBLK, 6], F32, tag="bn")
            mv = sc.tile([112, NBLK, 2], F32, tag="mv")
            for blk in range(NBLK):
                nc.vector.bn_stats(out=bn[:, blk, :], in_=ytm[:, blk, :])
                nc.vector.bn_aggr(out=mv[:, blk, :], in_=bn[:, blk, :])
            sg2 = sc.tile([112, 4], F32, tag="sg")
            nc.scalar.activation(out=sg2, in_=mv[:, :, 1], func=AF.Sqrt,
                                 bias=epsv)
            pk8 = sc.tile([112, 8], F32, tag="pk8")
            nc.vector.reciprocal(out=pk8[:, 0:4], in_=sg2)
            nc.vector.tensor_tensor(out=pk8[:, 4:8], in0=mv[:, :, 0],
                                    in1=pk8[:, 0:4], op=OP.mult)
            st["dg_r"] = _diag_prep(pk8[:, 0:4], "dg_r")
            st["dg_m"] = _diag_prep(pk8[:, 4:8], "dg_m")
            st["mv"], st["pk8"] = mv, pk8

        def front_b(st):
            """r2/mr broadcasts, yh, v (emitted after prior unit's attention
            matmuls so the LN2 chain has run)."""
            ybf, ytm = st["ybf"], st["ytm"]
            mv, pk8, pet = st["mv"], st["pk8"], st["pet"]
            prb = ps_st.tile([128, CH], F32, tag="st")
            _diag_bcast(prb, pk8[:, 0:4], ones112s, st["dg_r"])
            pmb = ps_st.tile([128, CH], F32, tag="st")
            _diag_bcast(pmb, pk8[:, 4:8], ones112m, st["dg_m"])
            r2b = wk.tile([128, CH], BF16, tag="r2b")
            nc.scalar.activation(out=r2b, in_=prb, func=AF.Copy)
            mrb = wk.tile([128, CH], BF16, tag="mrb")
            nc.scalar.activation(out=mrb, in_=pmb, func=AF.Copy)
            yh = wk.tile([128, 4, CH], BF16, tag="yh")
            for co in range(4):
                nc.vector.tensor_tensor(out=yh[:, co, :], in0=ybf[:, co, :],
                                        in1=r2b, op=OP.mult)
                if co < 2:
                    nc.vector.tensor_tensor(out=yh[:, co, :], in0=yh[:, co, :],
                                            in1=mrb, op=OP.add)
                else:
                    nc.gpsimd.tensor_add(out=yh[:, co, :], in0=yh[:, co, :],
                                         in1=mrb)
            v = wk.tile([112, NBLK, S_DIM], BF16, tag="v")
            for blk in range(NBLK):
                nc.vector.tensor_scalar(
                    out=v[:, blk, :], in0=ytm[:, blk, :],
                    scalar1=mv[:, blk, 0:1],
                    scalar2=pk8[:, blk:blk + 1],
                    op0=OP.subtract, op1=OP.mult)
                nc.vector.tensor_tensor(out=v[:, blk, :], in0=v[:, blk, :],
                                        in1=gb, op=OP.mult)
                nc.vector.tensor_tensor(out=v[:, blk, :], in0=v[:, blk, :],
                                        in1=pet[:, blk, :], op=OP.add)
            st["yh"], st["v"] = yh, v

        def mid(st):
            b, cols = st["b"], st["cols"]
            yh, ck_t, cq_t = st["yh"], st["ck_t"], st["cq_t"]
            # conv expansion
            xe = wk.tile([128, 6, CH], BF16, tag="xe")
            sq = wk.tile([128, 6, CH], BF16, tag="sq")
            for m in range(6):
                pxe = ps_mm.tile([128, CH], F32, tag="mm")
                for ik, (xt, kv) in enumerate(x_k):
                    nc.tensor.matmul(
                        pxe, xt[:kv, b, m * 128:(m + 1) * 128],
                        w1t[:kv, ik, cols], start=(ik == 0), stop=(ik == 1))
                if m % 2 == 0:
                    nc.vector.tensor_copy(out=xe[:, m, :], in_=pxe)
                    nc.scalar.square(out=sq[:, m, :], in_=xe[:, m, :])
                else:
                    nc.scalar.activation(out=xe[:, m, :], in_=pxe, func=AF.Copy)
                    nc.vector.tensor_tensor(out=sq[:, m, :], in0=xe[:, m, :],
                                            in1=xe[:, m, :], op=OP.mult)
            # LN1 stats: sum row via xbar trick, sumsq via ones-matmuls
            ps1 = ps_st.tile([1, CH], F32, tag="st")
            nc.tensor.matmul(ps1, xb0[:, b:b + 1], w1t[:, 0, cols],
                             start=True, stop=False)
            nc.tensor.matmul(ps1, xb1[:21, b:b + 1], w1t[:21, 1, cols],
                             start=False, stop=True)
            pq1 = ps_st.tile([1, CH], F32, tag="st")
            for m in range(6):
                nc.tensor.matmul(pq1, onesc, sq[:, m, :],
                                 start=(m == 0), stop=(m == 5))
            mrow1 = sc.tile([1, CH], BF16, tag="mrow1")
            nc.scalar.activation(out=mrow1, in_=ps1, func=AF.Copy,
                                 scale=-1.0 / T_DIM)
            # k projection first (yh ready from front_b; the LN1 row copies
            # run on V/S during these matmuls)
            kb = wk.tile([128, 4, CH], BF16, tag="kb")
            for oc in range(4):
                pk = ps_mm.tile([128, CH], F32, tag="mm")
                for kc in range(4):
                    nc.tensor.matmul(
                        pk, wkgt[:, kc, oc * 128:(oc + 1) * 128],
                        yh[:, kc, :], start=(kc == 0), stop=(kc == 3))
                nc.scalar.activation(out=kb[:, oc, :], in_=pk, func=AF.Copy)
                nc.vector.tensor_tensor(out=kb[:, oc, :], in0=kb[:, oc, :],
                                        in1=ck_t[:, oc, :], op=OP.add)
            _, sg1 = _stat_cols(ps1, pq1, T_DIM, epsb1)
            rcol1 = sc.tile([112, 4], F32, tag="rcol1")
            nc.vector.reciprocal(out=rcol1, in_=sg1)
            # q projection; evacuate raw via scalar so PSUM frees without
            # waiting on the r1 broadcast
            pqf = wk.tile([128, 4, CH], BF16, tag="pqf")
            for oc in range(4):
                pq = ps_mm.tile([128, CH], F32, tag="mm")
                nc.tensor.matmul(pq, uq[:, oc * 128:(oc + 1) * 128], mrow1,
                                 start=True, stop=False)
                for kc in range(6):
                    nc.tensor.matmul(
                        pq, wqgt[:, kc, oc * 128:(oc + 1) * 128],
                        xe[:, kc, :], start=False, stop=(kc == 5))
                nc.scalar.activation(out=pqf[:, oc, :], in_=pq, func=AF.Copy)
            # r1 broadcast (rcol1 chain finished during the q matmuls)
            pr1 = ps_st.tile([128, CH], F32, tag="st")
            _diag_bcast(pr1, rcol1, ones112q)
            r1b = wk.tile([128, CH], BF16, tag="r1b")
            nc.scalar.activation(out=r1b, in_=pr1, func=AF.Copy)
            pqb = wk.tile([128, 4, CH], BF16, tag="pqb")
            for oc in range(4):
                nc.vector.tensor_tensor(out=pqb[:, oc, :], in0=pqf[:, oc, :],
                                        in1=r1b, op=OP.mult)
                nc.vector.tensor_tensor(out=pqb[:, oc, :], in0=pqb[:, oc, :],
                                        in1=cq_t[:, oc, :], op=OP.add)
            st["pqb"], st["kb"] = pqb, kb

        def back(st):
            b, cols = st["b"], st["cols"]
            pqb, kb, v, ybf = st["pqb"], st["kb"], st["v"], st["ybf"]
            out_t = outp.tile([128, 4, CH], BF16, tag="out")

            def emit_psc(blk):
                tb = slice(blk * 112, (blk + 1) * 112)
                psc = ps_att.tile([112, 112], F32, tag="at")
                nc.tensor.matmul(psc, mA, mB, start=True, stop=False)
                for oc in range(4):
                    nc.tensor.matmul(psc, pqb[:, oc, tb], kb[:, oc, tb],
                                     start=False, stop=(oc == 3))
                e_t = att.tile([112, 112], BF16, tag="e_t")
                den = att.tile([112, 1], F32, tag="den")
                nc.scalar.activation(out=e_t, in_=psc, func=AF.Exp,
                                     accum_out=den)
                rden = att.tile([112, 1], F32, tag="rden")
                nc.vector.reciprocal(out=rden, in_=den)
                attn = att.tile([112, 112], BF16, tag="attn")
                nc.vector.tensor_scalar_mul(out=attn, in0=e_t, scalar1=rden)
                return attn

            def emit_av(blk, attn):
                tb = slice(blk * 112, (blk + 1) * 112)
                pat = ps_att.tile([112, 112], BF16, tag="at")
                nc.tensor.transpose(pat, attn, id112)
                attnT = att.tile([112, 112], BF16, tag="attnT")
                nc.vector.tensor_copy(out=attnT, in_=pat)
                pav = ps_mm.tile([128, 4, 112], F32, tag="mm")
                for co in range(4):
                    nc.tensor.matmul(pav[:, co, :],
                                     v[:, blk, co * 128:(co + 1) * 128],
                                     attnT, start=True, stop=True)
                nc.vector.tensor_copy(out=out_t[:, :, tb], in_=pav)
                nc.vector.tensor_tensor(out=out_t[:, :, tb],
                                        in0=out_t[:, :, tb],
                                        in1=ybf[:, :, tb], op=OP.add)

            attns = [None] * NBLK
            attns[0] = emit_psc(0)
            attns[1] = emit_psc(1)
            attns[2] = emit_psc(2)
            emit_av(0, attns[0])
            attns[3] = emit_psc(3)
            emit_av(1, attns[1])
            emit_av(2, attns[2])
            emit_av(3, attns[3])
            nc.sync.dma_start(out=dout[b, :, :, cols], in_=out_t)

        # ---- software-pipelined main loop over 14 (chunk, batch) units ----
        NU = NCHUNK * B_LOC
        states = [None] * NU
        chunk_consts = {}

        def do_load(u):
            ic, b = u // B_LOC, u % B_LOC
            if ic not in chunk_consts:
                cols = slice(ic * CH, (ic + 1) * CH)
                cq_t = io2.tile([128, 4, CH], BF16, tag="cq")
                nc.sync.dma_start(out=cq_t, in_=din["cq"][:, :, cols])
                ck_t = io2.tile([128, 4, CH], BF16, tag="ck")
                nc.sync.dma_start(out=ck_t, in_=din["ck"][:, :, cols])
                pet = io2.tile([112, NBLK, S_DIM], BF16, tag="pet")
                nc.sync.dma_start(out=pet, in_=din["petm"][ic])
                chunk_consts[ic] = (cq_t, ck_t, pet)
            cq_t, ck_t, pet = chunk_consts[ic]
            st = {"b": b, "ic": ic, "cols": slice(ic * CH, (ic + 1) * CH),
                  "cq_t": cq_t, "ck_t": ck_t, "pet": pet}
            states[u] = st
            phase_load(st)

        do_load(0)
        do_load(1)
        w1t = load("w1t", (128, 2, N_TOK), BF16)
        wkgt = load("wkgt", (128, 4, S_DIM), BF16)
        wqgt = load("wqgt", (128, 6, S_DIM), BF16)
        front_a(states[0])
        front_b(states[0])
        mid(states[0])
        for u in range(NU):
            if u + 2 < NU:
                do_load(u + 2)
            if u + 1 < NU:
                front_a(states[u + 1])
            back(states[u])
            if u + 1 < NU:
                front_b(states[u + 1])
                mid(states[u + 1])
    return nc


# ----------------------------------------------------------------------------
# Host-side preparation
# ----------------------------------------------------------------------------
def _make_const_inputs(W_conv1, b_conv1, ln1_g, ln1_b, ln2_g, ln2_b,
                       pe_wave, pe_spec, Wq, bq, Wk, bk):
    import ml_dtypes
    f = np.float32
    bf = ml_dtypes.bfloat16
    s = np.float32(S_DIM) ** np.float32(-0.25)
    sd1 = np.sqrt(np.float32(T_DIM))
    sd2 = np.sqrt(np.float32(S_DIM))

    w1t = np.zeros((128, 2, N_TOK), dtype=f)
    w1T = W_conv1.T.astype(f)
    w1t[:, 0, :] = w1T[:128]
    w1t[:21, 1, :] = w1T[128:]

    wqg = (Wq * ln1_g[None, :]).astype(f) * s
    wqgt = wqg.T.reshape(6, 128, S_DIM).transpose(1, 0, 2).copy()
    uq = (Wq @ ln1_g).astype(f)[None, :] * s

    pe_w = pe_wave.reshape(T_DIM, N_TOK).astype(f)
    cq = (Wq @ (ln1_b[:, None] + pe_w) + bq[:, None]).astype(f) * s
    cq = cq.reshape(4, 128, N_TOK).transpose(1, 0, 2).copy()

    wkg = (Wk * ln2_g[None, :]).astype(f) * s
    wkgt = wkg.T.reshape(4, 128, S_DIM).transpose(1, 0, 2).copy()

    pe2p = (pe_spec.reshape(S_DIM, N_TOK) + ln2_b[:, None]).astype(f)
    ck = ((Wk @ pe2p) + bk[:, None]).astype(f) * s
    ck = ck.reshape(4, 128, N_TOK).transpose(1, 0, 2).copy()

    petm = pe2p.T.reshape(NCHUNK, NBLK, 112, S_DIM).transpose(0, 2, 1, 3).copy()

    gbrow = ln2_g.astype(f)[None, :]
    gb = np.broadcast_to(gbrow, (112, S_DIM)).copy()

    u0 = np.zeros(112, f); u0[:56] = 1.0
    u1 = np.zeros(112, f); u1[56:] = 1.0
    mA = np.stack([np.ones(112, f), u0, u1])
    mB = np.stack([np.full(112, BIGNEG, f), -BIGNEG * u0, -BIGNEG * u1])

    return {
        "w1t": w1t.astype(bf), "wqgt": wqgt.astype(bf), "uq": uq.astype(bf),
        "cq": cq.astype(bf), "wkgt": wkgt.astype(bf), "ck": ck.astype(bf),
        "petm": petm.astype(bf), "gb": gb.astype(bf),
        "mA": mA.astype(bf), "mB": mB.astype(bf),
        "onesc": np.ones((128, 1), dtype=bf),
        "ones112s": np.full((112, 128), 1.0, dtype=bf),
        "ones112q": np.full((112, 128), sd1, dtype=bf),
        "ones112m": np.full((112, 128), -1.0, dtype=bf),
        "id112": np.eye(112, dtype=bf),
        "idf": np.eye(2, dtype=f),
    }


def _make_core_inputs(consts, x_shard, y_shard):
    import ml_dtypes
    bf = ml_dtypes.bfloat16
    x0 = x_shard[:, :128, :].transpose(1, 0, 2).astype(bf).copy()
    x1 = np.zeros((32, B_LOC, T_DIM), dtype=bf)
    x1[:21] = x_shard[:, 128:, :].transpose(1, 0, 2).astype(bf)
    y = y_shard.reshape(B_LOC, 4, 128, N_TOK).transpose(0, 2, 1, 3)
    y = np.ascontiguousarray(y).astype(bf)
    # token-major y: (B, 7, 112, 4, 512)
    ytm = y_shard.reshape(B_LOC, S_DIM, N_TOK).transpose(0, 2, 1)
    ytm = ytm.reshape(B_LOC, NCHUNK, NBLK, 112, S_DIM).transpose(0, 1, 3, 2, 4)
    ytm = np.ascontiguousarray(ytm).astype(bf)
    m = {"x0": x0, "x1": x1, "y": y, "ytm": ytm}
    m.update(consts)
    return m


_cached_nc = [None]


def kernel(x, y, W_conv1, b_conv1, ln1_g, ln1_b, ln2_g, ln2_b,
           pe_wave, pe_spec, Wq, bq, Wk, bk):
    _install_patch()
    from concourse.bass_utils import run_bass_kernel_spmd

    x = np.asarray(x, dtype=np.float32)
    y = np.asarray(y, dtype=np.float32)
    consts = _make_const_inputs(
        np.asarray(W_conv1, np.float32), np.asarray(b_conv1, np.float32),
        np.asarray(ln1_g, np.float32), np.asarray(ln1_b, np.float32),
        np.asarray(ln2_g, np.float32), np.asarray(ln2_b, np.float32),
        np.asarray(pe_wave, np.float32), np.asarray(pe_spec, np.float32),
        np.asarray(Wq, np.float32), np.asarray(bq, np.float32),
        np.asarray(Wk, np.float32), np.asarray(bk, np.float32))
    in_maps = [
        _make_core_inputs(consts, x[B_LOC * i:B_LOC * (i + 1)],
                          y[B_LOC * i:B_LOC * (i + 1)])
        for i in range(N_CORES)
    ]

    if _cached_nc[0] is None:
        _cached_nc[0] = _build_program()
    nc = _cached_nc[0]

    res = run_bass_kernel_spmd(nc, in_maps, core_ids=list(range(N_CORES)))
    outs = []
    for i in range(N_CORES):
        o = np.asarray(res.results[i]["out"], dtype=np.float32)
        outs.append(o.transpose(0, 2, 1, 3).reshape(B_LOC, S_DIM, H, W))
    return np.concatenate(outs, axis=0)


# revision 29
# speedup vs baseline: 1.0845x; 1.0845x over previous
"""Trainium2 Bass kernel for nn_Expand_36610301231376.

kernel(**inputs) takes the FULL unsharded inputs (as in reference.setup_inputs)
and returns the FULL (16, 512, 56, 56) float32 output.

Strategy: pure data parallel over batch B=16 across 8 NeuronCores (2 batches
per core); all parameters replicated. Per core, tokens (56x56=3136) are
processed channel-major in 7 chunks of 448 (8 image rows); row-local attention
runs on 2-row blocks of 112 tokens.

Key structure (v2):
- All elementwise traffic is bf16 (y, consts, output) for 2x/4x DVE modes;
  output is upcast to f32 on host.
- LayerNorm statistics: channel-dim sums via ones-matmuls into [1,448] PSUM
  rows, then tiny PE transposes to [112,4] column form so sqrt/reciprocal run
  parallel across partitions (the serial [1,448] reciprocal was ~3.7us each).
- LN gammas fold into the projection weights; LN1's r folds into a
  per-partition tensor_scalar on the score PSUM; the conv bias cancels in LN.
- The softmax mask is a rank-3 matmul accumulated into the score PSUM.
- v comes from a host-supplied token-major copy of y normalized with
  per-partition tensor_scalar ops (no PE transposes of ny); the positional-
  encoding contribution to the attention output is two extra const-lhsT
  matmuls accumulated into the same PSUM group.
"""
import sys

if "/opt/trn_rl_repo" not in sys.path:
    sys.path.insert(0, "/opt/trn_rl_repo")

import numpy as np
import orjson

# ----------------------------------------------------------------------------
# BIR post-pass: this container's walrus build supports only ONE sync-wait per
# instruction; split multi-wait instructions into single-wait NoOps.
# ----------------------------------------------------------------------------
_wcounter = [0]


def _split_block(instructions):
    out, changed = [], False
    for inst in instructions:
        si = inst.get("sync_info")
        waits = (si or {}).get("on_wait") or []
        if len(waits) > 1:
            changed = True
            for w in waits[:-1]:
                _wcounter[0] += 1
                nop = {
                    "engine": inst["engine"], "ins": [], "outs": [],
                    "name": f"I-wsplit-{_wcounter[0]}", "opcode": "NoOp",
                    "sync_info": {"on_update": [], "on_wait": [w]},
                }
                if "debug" in inst:
                    nop["debug"] = inst["debug"]
                out.append(nop)
            si["on_wait"] = [waits[-1]]
        out.append(inst)
    return out, changed


def _split_multi_waits_json(bir_json: bytes) -> bytes:
    m = orjson.loads(bir_json)
    changed = False
    for fn in m.get("functions", []):
        for blk in fn.get("blocks", []):
            insts = blk.get("instructions")
            if insts:
                blk["instructions"], ch = _split_block(insts)
                changed = changed or ch
    return orjson.dumps(m) if changed else bir_json


def _install_patch():
    import concourse.bass as bass

    if getattr(bass.Bass, "_wait_split_installed", False):
        return
    orig = bass.Bass.to_json_bytes

    def to_json_bytes(self):
        return _split_multi_waits_json(orig(self))

    bass.Bass.to_json_bytes = to_json_bytes
    bass.Bass._wait_split_installed = True


# ----------------------------------------------------------------------------
# Problem constants (hardcoded from the problem spec)
# ----------------------------------------------------------------------------
B = 16
N_CORES = 8
B_LOC = B // N_CORES
T_LEN, T_DIM = 149, 768
H = W = 56
S_DIM = 512
N_TOK = H * W           # 3136
CH = 448                # tokens per chunk (8 image rows)
NCHUNK = N_TOK // CH    # 7
NBLK = CH // 112        # 4 two-row attention blocks per chunk
EPS = 1e-5
BIGNEG = -1e30


# ----------------------------------------------------------------------------
# Device program
# ----------------------------------------------------------------------------
def _build_program():
    import concourse.bass as bass
    import concourse.tile as tile
    from concourse import mybir

    F32 = mybir.dt.float32
    BF16 = mybir.dt.bfloat16
    AF = mybir.ActivationFunctionType
    OP = mybir.AluOpType

    nc = bass.Bass(trn_type="TRN2", target_bir_lowering=False, debug=False)
    din = {}
    for name, shape, dt_ in [
        ("x0", (128, B_LOC, T_DIM), BF16), ("x1", (32, B_LOC, T_DIM), BF16),
        ("w1t", (128, 2, N_TOK), BF16),
        ("wqgt", (128, 6, S_DIM), BF16), ("uq", (1, S_DIM), BF16),
        ("cq", (128, 4, N_TOK), BF16),
        ("wkgt", (128, 4, S_DIM), BF16), ("ck", (128, 4, N_TOK), BF16),
        ("gb", (112, S_DIM), BF16),
        ("mA", (3, 112), BF16), ("mB", (3, 112), BF16),
        ("onesc", (128, 1), BF16), ("ones112s", (112, 128), BF16),
        ("ones112q", (112, 128), BF16), ("ones112m", (112, 128), BF16),
        ("id112", (112, 112), BF16), ("idf", (2, 2), F32),
        ("y", (B_LOC, 128, 4, N_TOK), BF16),
        ("ytm", (B_LOC, NCHUNK, 112, NBLK, S_DIM), BF16),
        ("petm", (NCHUNK, 112, NBLK, S_DIM), BF16),
    ]:
        din[name] = nc.dram_tensor(name, list(shape), dt_, kind="ExternalInput").ap()
    dout = nc.dram_tensor("out", [B_LOC, 128, 4, N_TOK], BF16,
                          kind="ExternalOutput").ap()

    from contextlib import ExitStack

    with nc.allow_low_precision(reason="bf16 ops, fp32 accumulate"), \
         tile.TileContext(nc) as tc, ExitStack() as ctx:
        singles = ctx.enter_context(tc.tile_pool(name="singles", bufs=1))
        io3 = ctx.enter_context(tc.tile_pool(name="io3", bufs=3))
        io2 = ctx.enter_context(tc.tile_pool(name="io2", bufs=2))
        outp = ctx.enter_context(tc.tile_pool(name="outp", bufs=2))
        wk = ctx.enter_context(tc.tile_pool(name="wk", bufs=2))
        sc = ctx.enter_context(tc.tile_pool(name="sc", bufs=2))
        att = ctx.enter_context(tc.tile_pool(name="att", bufs=3))
        ps_mm = ctx.enter_context(tc.tile_pool(name="ps_mm", bufs=3, space="PSUM"))
        ps_st = ctx.enter_context(tc.tile_pool(name="ps_st", bufs=2, space="PSUM"))
        ps_sm = ctx.enter_context(tc.tile_pool(name="ps_sm", bufs=1, space="PSUM"))
        ps_att = ctx.enter_context(tc.tile_pool(name="ps_att", bufs=2, space="PSUM"))

        def load(name, shape, dt_):
            t = singles.tile(list(shape), dt_, tag=name)
            nc.sync.dma_start(out=t, in_=din[name])
            return t

        x0 = load("x0", (128, B_LOC, T_DIM), BF16)
        x1 = load("x1", (32, B_LOC, T_DIM), BF16)
        w1t = load("w1t", (128, 2, N_TOK), BF16)
        wqgt = load("wqgt", (128, 6, S_DIM), BF16)
        uq = load("uq", (1, S_DIM), BF16)
        wkgt = load("wkgt", (128, 4, S_DIM), BF16)
        gb = load("gb", (112, S_DIM), BF16)
        mA = load("mA", (3, 112), BF16)
        mB = load("mB", (3, 112), BF16)
        onesc = load("onesc", (128, 1), BF16)
        ones112s = load("ones112s", (112, 128), BF16)
        ones112q = load("ones112q", (112, 128), BF16)
        ones112m = load("ones112m", (112, 128), BF16)
        id112 = load("id112", (112, 112), BF16)
        idf = load("idf", (2, 2), F32)

        epsb1 = singles.tile([112, 1], F32, tag="epsb1")
        nc.vector.memset(epsb1, T_DIM * EPS)
        epsv = singles.tile([112, 1], F32, tag="epsv")
        nc.vector.memset(epsv, EPS)

        x_k = [(x0, 128), (x1, 21)]

        # per-batch column sums of x (for the LN1 sum row): xbar_l = sum_d x[l,d]
        junk0 = singles.tile([128, T_DIM], BF16, tag="junk0")
        junk1 = singles.tile([32, T_DIM], BF16, tag="junk1")
        xb0f = singles.tile([128, B_LOC], F32, tag="xb0f")
        xb1f = singles.tile([32, B_LOC], F32, tag="xb1f")
        for b in range(B_LOC):
            nc.scalar.activation(out=junk0, in_=x0[:, b, :], func=AF.Copy,
                                 accum_out=xb0f[:, b:b + 1])
            nc.scalar.activation(out=junk1, in_=x1[:, b, :], func=AF.Copy,
                                 accum_out=xb1f[:, b:b + 1])
        xb0 = singles.tile([128, B_LOC], BF16, tag="xb0")
        nc.vector.tensor_copy(out=xb0, in_=xb0f)
        xb1 = singles.tile([32, B_LOC], BF16, tag="xb1")
        nc.vector.tensor_copy(out=xb1, in_=xb1f)

        def phase_load(st):
            b, ic = st["b"], st["ic"]
            ybf = io3.tile([128, 4, CH], BF16, tag="ybf")
            nc.sync.dma_start(out=ybf, in_=din["y"][b, :, :, st["cols"]])
            ytm = io3.tile([112, NBLK, S_DIM], BF16, tag="ytm")
            nc.sync.dma_start(out=ytm, in_=din["ytm"][b, ic])
            st["ybf"], st["ytm"] = ybf, ytm

        def _stat_cols(ps_s, ps_q, d, epsb):
            """[1,CH] PSUM sum/sumsq rows -> column form [112,4] stats."""
            rows_s = sc.tile([1, CH], F32, tag="rows_s")
            nc.vector.tensor_copy(out=rows_s, in_=ps_s)
            rows_q = sc.tile([1, CH], F32, tag="rows_q")
            nc.scalar.activation(out=rows_q, in_=ps_q, func=AF.Copy)
            ps_c = ps_sm.tile([112, 8], F32, tag="col")
            id1 = idf[0:1, 0:1]
            for blk in range(NBLK):
                tb = slice(blk * 112, (blk + 1) * 112)
                nc.tensor.transpose(ps_c[:, blk:blk + 1], rows_s[:, tb], id1)
                nc.tensor.transpose(ps_c[:, 4 + blk:5 + blk], rows_q[:, tb], id1)
            scol = ps_c[:, 0:4]
            qcol = ps_c[:, 4:8]
            m2c = sc.tile([112, 4], F32, tag="m2c")
            nc.scalar.activation(out=m2c, in_=scol, func=AF.Square,
                                 scale=float(1.0 / np.sqrt(d)))
            uc = sc.tile([112, 4], F32, tag="uc")
            nc.vector.tensor_tensor(out=uc, in0=qcol, in1=m2c, op=OP.subtract)
            sg = sc.tile([112, 4], F32, tag="sg")
            nc.scalar.activation(out=sg, in_=uc, func=AF.Sqrt, bias=epsb)
            return scol, sg

        def _diag_bcast(prb, colap, ones_t):
            for blk in range(NBLK):
                tb = slice(blk * 112, (blk + 1) * 112)
                dgr = sc.tile([112, 112], BF16, tag="dgr")
                nc.vector.tensor_scalar(
                    out=dgr, in0=id112, scalar1=colap[:, blk:blk + 1],
                    scalar2=None, op0=OP.mult)
                nc.tensor.matmul(prb[:, tb], ones_t, dgr,
                                 start=True, stop=True)

        def front_a(st):
            """LN2 stats via bn_stats/bn_aggr on token-major y (free-axis)."""
            ytm = st["ytm"]
            bn = sc.tile([112, NBLK, 6], F32, tag="bn")
            mv = sc.tile([112, NBLK, 2], F32, tag="mv")
            for blk in range(NBLK):
                nc.vector.bn_stats(out=bn[:, blk, :], in_=ytm[:, blk, :])
                nc.vector.bn_aggr(out=mv[:, blk, :], in_=bn[:, blk, :])
            sg2 = sc.tile([112, 4], F32, tag="sg")
            nc.scalar.activation(out=sg2, in_=mv[:, :, 1], func=AF.Sqrt,
                                 bias=epsv)
            pk8 = sc.tile([112, 8], F32, tag="pk8")
            nc.vector.reciprocal(out=pk8[:, 0:4], in_=sg2)
            nc.vector.tensor_tensor(out=pk8[:, 4:8], in0=mv[:, :, 0],
                                    in1=pk8[:, 0:4], op=OP.mult)
            st["mv"], st["pk8"] = mv, pk8

        def front_b(st):
            """r2/mr broadcasts, yh, v (emitted after prior unit's attention
            matmuls so the LN2 chain has run)."""
            ybf, ytm = st["ybf"], st["ytm"]
            mv, pk8, pet = st["mv"], st["pk8"], st["pet"]
            prb = ps_st.tile([128, CH], F32, tag="st")
            _diag_bcast(prb, pk8[:, 0:4], ones112s)
            pmb = ps_st.tile([128, CH], F32, tag="st")
            _diag_bcast(pmb, pk8[:, 4:8], ones112m)
            r2b = wk.tile([128, CH], BF16, tag="r2b")
            nc.scalar.activation(out=r2b, in_=prb, func=AF.Copy)
            mrb = wk.tile([128, CH], BF16, tag="mrb")
            nc.scalar.activation(out=mrb, in_=pmb, func=AF.Copy)
            yh = wk.tile([128, 4, CH], BF16, tag="yh")
            for co in range(4):
                nc.vector.tensor_tensor(out=yh[:, co, :], in0=ybf[:, co, :],
                                        in1=r2b, op=OP.mult)
                if co < 2:
                    nc.vector.tensor_tensor(out=yh[:, co, :], in0=yh[:, co, :],
                                            in1=mrb, op=OP.add)
                else:
                    nc.gpsimd.tensor_add(out=yh[:, co, :], in0=yh[:, co, :],
                                         in1=mrb)
            v = wk.tile([112, NBLK, S_DIM], BF16, tag="v")
            for blk in range(NBLK):
                nc.vector.tensor_scalar(
                    out=v[:, blk, :], in0=ytm[:, blk, :],
                    scalar1=mv[:, blk, 0:1],
                    scalar2=pk8[:, blk:blk + 1],
                    op0=OP.subtract, op1=OP.mult)
                nc.vector.tensor_tensor(out=v[:, blk, :], in0=v[:, blk, :],
                                        in1=gb, op=OP.mult)
                nc.vector.tensor_tensor(out=v[:, blk, :], in0=v[:, blk, :],
                                        in1=pet[:, blk, :], op=OP.add)
            st["yh"], st["v"] = yh, v

        def mid(st):
            b, cols = st["b"], st["cols"]
            yh, ck_t, cq_t = st["yh"], st["ck_t"], st["cq_t"]
            # conv expansion
            xe = wk.tile([128, 6, CH], BF16, tag="xe")
            sq = wk.tile([128, 6, CH], BF16, tag="sq")
            for m in range(6):
                pxe = ps_mm.tile([128, CH], F32, tag="mm")
                for ik, (xt, kv) in enumerate(x_k):
                    nc.tensor.matmul(
                        pxe, xt[:kv, b, m * 128:(m + 1) * 128],
                        w1t[:kv, ik, cols], start=(ik == 0), stop=(ik == 1))
                if m % 2 == 0:
                    nc.vector.tensor_copy(out=xe[:, m, :], in_=pxe)
                    nc.scalar.square(out=sq[:, m, :], in_=xe[:, m, :])
                else:
                    nc.scalar.activation(out=xe[:, m, :], in_=pxe, func=AF.Copy)
                    nc.vector.tensor_tensor(out=sq[:, m, :], in0=xe[:, m, :],
                                            in1=xe[:, m, :], op=OP.mult)
            # LN1 stats: sum row via xbar trick, sumsq via ones-matmuls
            ps1 = ps_st.tile([1, CH], F32, tag="st")
            nc.tensor.matmul(ps1, xb0[:, b:b + 1], w1t[:, 0, cols],
                             start=True, stop=False)
            nc.tensor.matmul(ps1, xb1[:21, b:b + 1], w1t[:21, 1, cols],
                             start=False, stop=True)
            pq1 = ps_st.tile([1, CH], F32, tag="st")
            for m in range(6):
                nc.tensor.matmul(pq1, onesc, sq[:, m, :],
                                 start=(m == 0), stop=(m == 5))
            mrow1 = sc.tile([1, CH], BF16, tag="mrow1")
            nc.scalar.activation(out=mrow1, in_=ps1, func=AF.Copy,
                                 scale=-1.0 / T_DIM)
            # k projection first (yh ready from front_b; the LN1 row copies
            # run on V/S during these matmuls)
            kb = wk.tile([128, 4, CH], BF16, tag="kb")
            for oc in range(4):
                pk = ps_mm.tile([128, CH], F32, tag="mm")
                for kc in range(4):
                    nc.tensor.matmul(
                        pk, wkgt[:, kc, oc * 128:(oc + 1) * 128],
                        yh[:, kc, :], start=(kc == 0), stop=(kc == 3))
                nc.scalar.activation(out=kb[:, oc, :], in_=pk, func=AF.Copy)
                nc.vector.tensor_tensor(out=kb[:, oc, :], in0=kb[:, oc, :],
                                        in1=ck_t[:, oc, :], op=OP.add)
            _, sg1 = _stat_cols(ps1, pq1, T_DIM, epsb1)
            rcol1 = sc.tile([112, 4], F32, tag="rcol1")
            nc.vector.reciprocal(out=rcol1, in_=sg1)
            # q projection; evacuate raw via scalar so PSUM frees without
            # waiting on the r1 broadcast
            pqf = wk.tile([128, 4, CH], BF16, tag="pqf")
            for oc in range(4):
                pq = ps_mm.tile([128, CH], F32, tag="mm")
                nc.tensor.matmul(pq, uq[:, oc * 128:(oc + 1) * 128], mrow1,
                                 start=True, stop=False)
                for kc in range(6):
                    nc.tensor.matmul(
                        pq, wqgt[:, kc, oc * 128:(oc + 1) * 128],
                        xe[:, kc, :], start=False, stop=(kc == 5))
                nc.scalar.activation(out=pqf[:, oc, :], in_=pq, func=AF.Copy)
            # r1 broadcast (rcol1 chain finished during the q matmuls)
            pr1 = ps_st.tile([128, CH], F32, tag="st")
            _diag_bcast(pr1, rcol1, ones112q)
            r1b = wk.tile([128, CH], BF16, tag="r1b")
            nc.scalar.activation(out=r1b, in_=pr1, func=AF.Copy)
            pqb = wk.tile([128, 4, CH], BF16, tag="pqb")
            for oc in range(4):
                nc.vector.tensor_tensor(out=pqb[:, oc, :], in0=pqf[:, oc, :],
                                        in1=r1b, op=OP.mult)
                nc.vector.tensor_tensor(out=pqb[:, oc, :], in0=pqb[:, oc, :],
                                        in1=cq_t[:, oc, :], op=OP.add)
            st["pqb"], st["kb"] = pqb, kb

        def back(st):
            b, cols = st["b"], st["cols"]
            pqb, kb, v, ybf = st["pqb"], st["kb"], st["v"], st["ybf"]
            out_t = outp.tile([128, 4, CH], BF16, tag="out")

            def emit_psc(blk):
                tb = slice(blk * 112, (blk + 1) * 112)
                psc = ps_att.tile([112, 112], F32, tag="at")
                nc.tensor.matmul(psc, mA, mB, start=True, stop=False)
                for oc in range(4):
                    nc.tensor.matmul(psc, pqb[:, oc, tb], kb[:, oc, tb],
                                     start=False, stop=(oc == 3))
                e_t = att.tile([112, 112], BF16, tag="e_t")
                den = att.tile([112, 1], F32, tag="den")
                nc.scalar.activation(out=e_t, in_=psc, func=AF.Exp,
                                     accum_out=den)
                rden = att.tile([112, 1], F32, tag="rden")
                nc.vector.reciprocal(out=rden, in_=den)
                attn = att.tile([112, 112], BF16, tag="attn")
                nc.vector.tensor_scalar_mul(out=attn, in0=e_t, scalar1=rden)
                return attn

            def emit_av(blk, attn):
                tb = slice(blk * 112, (blk + 1) * 112)
                pat = ps_att.tile([112, 112], BF16, tag="at")
                nc.tensor.transpose(pat, attn, id112)
                attnT = att.tile([112, 112], BF16, tag="attnT")
                nc.vector.tensor_copy(out=attnT, in_=pat)
                pav = ps_mm.tile([128, 4, 112], F32, tag="mm")
                for co in range(4):
                    nc.tensor.matmul(pav[:, co, :],
                                     v[:, blk, co * 128:(co + 1) * 128],
                                     attnT, start=True, stop=True)
                nc.scalar.activation(out=out_t[:, :, tb], in_=pav,
                                     func=AF.Copy)
                nc.vector.tensor_tensor(out=out_t[:, :, tb],
                                        in0=out_t[:, :, tb],
                                        in1=ybf[:, :, tb], op=OP.add)

            attns = [None] * NBLK
            attns[0] = emit_psc(0)
            attns[1] = emit_psc(1)
            attns[2] = emit_psc(2)
            attns[3] = emit_psc(3)
            emit_av(0, attns[0])
            emit_av(1, attns[1])
            emit_av(2, attns[2])
            emit_av(3, attns[3])
            nc.sync.dma_start(out=dout[b, :, :, cols], in_=out_t)

        # ---- software-pipelined main loop over 14 (chunk, batch) units ----
        NU = NCHUNK * B_LOC
        states = [None] * NU
        chunk_consts = {}

        def do_load(u):
            ic, b = u // B_LOC, u % B_LOC
            if ic not in chunk_consts:
                cols = slice(ic * CH, (ic + 1) * CH)
                cq_t = io2.tile([128, 4, CH], BF16, tag="cq")
                nc.sync.dma_start(out=cq_t, in_=din["cq"][:, :, cols])
                ck_t = io2.tile([128, 4, CH], BF16, tag="ck")
                nc.sync.dma_start(out=ck_t, in_=din["ck"][:, :, cols])
                pet = io2.tile([112, NBLK, S_DIM], BF16, tag="pet")
                nc.sync.dma_start(out=pet, in_=din["petm"][ic])
                chunk_consts[ic] = (cq_t, ck_t, pet)
            cq_t, ck_t, pet = chunk_consts[ic]
            st = {"b": b, "ic": ic, "cols": slice(ic * CH, (ic + 1) * CH),
                  "cq_t": cq_t, "ck_t": ck_t, "pet": pet}
            states[u] = st
            phase_load(st)

        do_load(0)
        do_load(1)
        front_a(states[0])
        front_b(states[0])
        mid(states[0])
        for u in range(NU):
            if u + 2 < NU:
                do_load(u + 2)
            if u + 1 < NU:
                front_a(states[u + 1])
            back(states[u])
            if u + 1 < NU:
                front_b(states[u + 1])
                mid(states[u + 1])
    return nc


# ----------------------------------------------------------------------------
# Host-side preparation
# ----------------------------------------------------------------------------
def _make_const_inputs(W_conv1, b_conv1, ln1_g, ln1_b, ln2_g, ln2_b,
                       pe_wave, pe_spec, Wq, bq, Wk, bk):
    import ml_dtypes
    f = np.float32
    bf = ml_dtypes.bfloat16
    s = np.float32(S_DIM) ** np.float32(-0.25)
    sd1 = np.sqrt(np.float32(T_DIM))
    sd2 = np.sqrt(np.float32(S_DIM))

    w1t = np.zeros((128, 2, N_TOK), dtype=f)
    w1T = W_conv1.T.astype(f)
    w1t[:, 0, :] = w1T[:128]
    w1t[:21, 1, :] = w1T[128:]

    wqg = (Wq * ln1_g[None, :]).astype(f) * s
    wqgt = wqg.T.reshape(6, 128, S_DIM).transpose(1, 0, 2).copy()
    uq = (Wq @ ln1_g).astype(f)[None, :] * s

    pe_w = pe_wave.reshape(T_DIM, N_TOK).astype(f)
    cq = (Wq @ (ln1_b[:, None] + pe_w) + bq[:, None]).astype(f) * s
    cq = cq.reshape(4, 128, N_TOK).transpose(1, 0, 2).copy()

    wkg = (Wk * ln2_g[None, :]).astype(f) * s
    wkgt = wkg.T.reshape(4, 128, S_DIM).transpose(1, 0, 2).copy()

    pe2p = (pe_spec.reshape(S_DIM, N_TOK) + ln2_b[:, None]).astype(f)
    ck = ((Wk @ pe2p) + bk[:, None]).astype(f) * s
    ck = ck.reshape(4, 128, N_TOK).transpose(1, 0, 2).copy()

    petm = pe2p.T.reshape(NCHUNK, NBLK, 112, S_DIM).transpose(0, 2, 1, 3).copy()

    gbrow = ln2_g.astype(f)[None, :]
    gb = np.broadcast_to(gbrow, (112, S_DIM)).copy()

    u0 = np.zeros(112, f); u0[:56] = 1.0
    u1 = np.zeros(112, f); u1[56:] = 1.0
    mA = np.stack([np.ones(112, f), u0, u1])
    mB = np.stack([np.full(112, BIGNEG, f), -BIGNEG * u0, -BIGNEG * u1])

    return {
        "w1t": w1t.astype(bf), "wqgt": wqgt.astype(bf), "uq": uq.astype(bf),
        "cq": cq.astype(bf), "wkgt": wkgt.astype(bf), "ck": ck.astype(bf),
        "petm": petm.astype(bf), "gb": gb.astype(bf),
        "mA": mA.astype(bf), "mB": mB.astype(bf),
        "onesc": np.ones((128, 1), dtype=bf),
        "ones112s": np.full((112, 128), 1.0, dtype=bf),
        "ones112q": np.full((112, 128), sd1, dtype=bf),
        "ones112m": np.full((112, 128), -1.0, dtype=bf),
        "id112": np.eye(112, dtype=bf),
        "idf": np.eye(2, dtype=f),
    }


def _make_core_inputs(consts, x_shard, y_shard):
    import ml_dtypes
    bf = ml_dtypes.bfloat16
    x0 = x_shard[:, :128, :].transpose(1, 0, 2).astype(bf).copy()
    x1 = np.zeros((32, B_LOC, T_DIM), dtype=bf)
    x1[:21] = x_shard[:, 128:, :].transpose(1, 0, 2).astype(bf)
    y = y_shard.reshape(B_LOC, 4, 128, N_TOK).transpose(0, 2, 1, 3)
    y = np.ascontiguousarray(y).astype(bf)
    # token-major y: (B, 7, 112, 4, 512)
    ytm = y_shard.reshape(B_LOC, S_DIM, N_TOK).transpose(0, 2, 1)
    ytm = ytm.reshape(B_LOC, NCHUNK, NBLK, 112, S_DIM).transpose(0, 1, 3, 2, 4)
    ytm = np.ascontiguousarray(ytm).astype(bf)
    m = {"x0": x0, "x1": x1, "y": y, "ytm": ytm}
    m.update(consts)
    return m


_cached_nc = [None]


def kernel(x, y, W_conv1, b_conv1, ln1_g, ln1_b, ln2_g, ln2_b,
           pe_wave, pe_spec, Wq, bq, Wk, bk):
    _install_patch()
    from concourse.bass_utils import run_bass_kernel_spmd

    x = np.asarray(x, dtype=np.float32)
    y = np.asarray(y, dtype=np.float32)
    consts = _make_const_inputs(
        np.asarray(W_conv1, np.float32), np.asarray(b_conv1, np.float32),
        np.asarray(ln1_g, np.float32), np.asarray(ln1_b, np.float32),
        np.asarray(ln2_g, np.float32), np.asarray(ln2_b, np.float32),
        np.asarray(pe_wave, np.float32), np.asarray(pe_spec, np.float32),
        np.asarray(Wq, np.float32), np.asarray(bq, np.float32),
        np.asarray(Wk, np.float32), np.asarray(bk, np.float32))
    in_maps = [
        _make_core_inputs(consts, x[B_LOC * i:B_LOC * (i + 1)],
                          y[B_LOC * i:B_LOC * (i + 1)])
        for i in range(N_CORES)
    ]

    if _cached_nc[0] is None:
        _cached_nc[0] = _build_program()
    nc = _cached_nc[0]

    res = run_bass_kernel_spmd(nc, in_maps, core_ids=list(range(N_CORES)))
    outs = []
    for i in range(N_CORES):
        o = np.asarray(res.results[i]["out"], dtype=np.float32)
        outs.append(o.transpose(0, 2, 1, 3).reshape(B_LOC, S_DIM, H, W))
    return np.concatenate(outs, axis=0)
